# revision 55
# baseline (speedup 1.0000x reference)
"""DualLSTM Trainium2 kernel (8-core SPMD).

Strategy:
  - Embedding gather, gate-input projections (gx), fc1, fc2 run as tiled
    bf16 matmuls on every core; fc2 (the 134-GFLOP vocab projection) is
    sharded column-wise (vocab) across the 8 cores.
  - The 2047-step sequential dual-LSTM recurrence is replicated on all
    cores (it is a serial matvec chain; replication avoids per-step
    cross-core synchronization).  Per step the two shared matvecs
    u_en = W_hh_en @ h_en and u_cn = W_hh_cn @ h_cn run on the tensor
    engine with h as the stationary operand and the (bf16, SBUF-resident)
    weights streaming, accumulating in PSUM [1, 512] chunks which are
    evicted + partition-scattered into a [128, 64] gate-major tile for
    the vectorized gate nonlinearities.
  - All matmul inputs in bf16 (verified: |err|_inf / |out|_inf ~ 1.2e-3
    vs the fp32 reference), accumulation in fp32.
"""

import os
from contextlib import ExitStack

import numpy as np
import ml_dtypes

import concourse.bass as bass
import concourse.tile as tile
import concourse.mybir as mybir
from concourse import bacc
from concourse.bass import ds, ts, IndirectOffsetOnAxis
from concourse.bass_utils import run_bass_kernel_spmd
from concourse.kernels.tile_matmul import matmul_tile_kernel

BF16 = ml_dtypes.bfloat16
F32 = mybir.dt.float32
BF = mybir.dt.bfloat16
I32 = mybir.dt.int32

V, E, H, S = 32000, 512, 1024, 2048
T = S - 1            # 2047 recurrence steps
TP = S               # padded sequence dim (2048) for the dense matmuls
P = 128
HC = H // P          # 8 h-chunks
NCORES = 8
VS = V // NCORES     # 4000 real vocab columns per core
VSP = 4096           # padded vocab shard
NSTEPS = int(os.environ.get("DUAL_LSTM_STEPS", T))  # trim for smoke tests
SKIP = set(os.environ.get("DUAL_SKIP", "").split(","))  # phase bisect (timing only)
KPRUNE = int(os.environ.get("DUAL_KPRUNE", "8"))  # k-chunks per matvec (timing only)
NO_DMA = os.environ.get("DUAL_NO_DMA", "") == "1"      # timing bisect only
NO_EVICT = os.environ.get("DUAL_NO_EVICT", "") == "1"  # timing bisect only
NO_CHAIN = os.environ.get("DUAL_NO_CHAIN", "") == "1"  # timing bisect only

AF = mybir.ActivationFunctionType
OP = mybir.AluOpType
UNROLL = 4           # steps per hardware-loop iteration

# Route Copy/Identity to the 'sigmoid_and_others' ACT table set (which also
# holds Sigmoid/Tanh/Relu) so the per-step PSUM evictions on the scalar
# engine never force an activation-table reload (1.3us each) inside the
# recurrence loop.  We only shrink the *selection* map; the chosen set
# genuinely contains copy/identity, so numerics are unchanged.
_orig_gat = bacc.get_activation_tables


def _gat_pinned(arch):
    tables = _orig_gat(arch)
    for name, fns in tables.items():
        if name != "sigmoid_and_others":
            fns.discard(AF.Copy)
            fns.discard(AF.Identity)
    return tables


bacc.get_activation_tables = _gat_pinned


def _perm():
    """Packed gate-column order.

    Column c = n*512 + p*4 + mm maps to m-tile m = n*4+mm (slot order
    [i f o g], H-chunk-major within slot) at partition p, so each PSUM
    [1,512] chunk n scatters contiguously into u_sb[:, 4n:4n+4].
    perm[c] = original row in the 4H gate dimension."""
    og = np.array([0, 1, 3, 2])  # slot -> original gate index (i,f,g,o order)
    c = np.arange(4 * H)
    n, r = c // 512, c % 512
    p, mm = r // 4, r % 4
    m = n * 4 + mm
    return og[m // HC] * H + (m % HC) * P + p


def _pack_whh(W):  # [4H, H] -> [128, HC, 4H] (lhs-streamed, bf16)
    Wp = W[_perm()]                       # [4096, 1024]
    return np.ascontiguousarray(
        Wp.T.reshape(HC, P, 4 * H).transpose(1, 0, 2)).astype(BF16)


def _pack_wih(W_cn, W_en):
    """[E, 8192] with column e = p*64 + cell*32 + m so that one step's
    gx row [p, 64] is contiguous per partition and matches u_sb layout."""
    og = np.array([0, 1, 3, 2])
    e = np.arange(2 * 4 * H)
    p, c = e // 64, e % 64
    cell, m = c // 32, c % 32
    rows = og[m // HC] * H + (m % HC) * P + p
    Wb = np.stack([np.asarray(W_cn, np.float32), np.asarray(W_en, np.float32)])
    return np.ascontiguousarray(Wb[cell, rows, :].T).astype(BF16)


def build(nsteps=NSTEPS, skip=None):
    global SKIP
    SKIP = set(skip) if skip is not None else set(
        os.environ.get("DUAL_SKIP", "").split(","))
    # Bacc (not raw Bass): its compile() pass legalizes multi-wait
    # instructions for walrus (nop-fusion / wait splitting).
    nc = bacc.Bacc(None, target_bir_lowering=False, debug=False)

    # ---- kernel I/O ----
    sent = nc.dram_tensor("sent", [S], I32, kind="ExternalInput").ap()
    emb = nc.dram_tensor("emb", [V, E], BF, kind="ExternalInput").ap()
    whh = nc.dram_tensor("whh", [P, HC, 2 * 4 * H], BF, kind="ExternalInput").ap()
    wih = nc.dram_tensor("wih", [E, 2 * 4 * H], BF, kind="ExternalInput").ap()
    maskb = nc.dram_tensor("maskb", [P, TP], F32, kind="ExternalInput").ap()
    w1t = nc.dram_tensor("w1t", [H, H], BF, kind="ExternalInput").ap()
    b1p = nc.dram_tensor("b1p", [P, HC], F32, kind="ExternalInput").ap()
    w2t = nc.dram_tensor("w2t", [H, VSP], BF, kind="ExternalInput").ap()
    b2p = nc.dram_tensor("b2p", [P, VSP], F32, kind="ExternalInput").ap()
    ident = nc.dram_tensor("ident", [16, 16], F32, kind="ExternalInput").ap()
    out = nc.dram_tensor("out", [TP, VSP], F32, kind="ExternalOutput").ap()

    # ---- DRAM intermediates ----
    x_d = nc.dram_tensor("x_d", [S, E], BF).ap()
    gxs = nc.dram_tensor("gxs", [TP, P, 8 * HC], BF).ap()  # seq-major gx
    outst = nc.dram_tensor("outst", [H, TP], BF).ap()
    hidt = nc.dram_tensor("hidt", [H, TP], BF).ap()

    # ================= phase A: embedding gather =================
    if "A" not in SKIP:
      with tile.TileContext(nc) as tc:
        with tc.tile_pool(name="gather", bufs=1) as gp, \
             tc.tile_pool(name="gidx", bufs=1) as gip:
            idx = gip.tile([P, S // P], I32)
            nc.gpsimd.dma_start(idx[:], sent.rearrange("(j p) -> p j", p=P))
            for j in range(S // P):
                xg = gp.tile([P, E], BF, tag=f"xg{j}")
                nc.gpsimd.indirect_dma_start(
                    out=xg[:], out_offset=None, in_=emb[:],
                    in_offset=IndirectOffsetOnAxis(ap=idx[:, j:j + 1], axis=0))
                nc.gpsimd.dma_start(x_d[ts(j, P)], xg[:])

    # ============ phase B: gx[t, :] = x[t] @ wih  (seq-major) ======
    if "B" not in SKIP:
      with tile.TileContext(nc) as tc:
        with ExitStack() as c2:
            matmul_tile_kernel(
                tc,
                kxm_ap=x_d,            # [S, E] -> transposed to [E, S]
                kxn_ap=wih,            # [E, 8192]
                mxn_ap=gxs.rearrange("t p c -> t (p c)"),  # [2048, 8192]
                transpose_kxm=True,
            )

    # ================= phase C: recurrence =================
    if "C" not in SKIP:
      with tile.TileContext(nc) as tc:
        cr = ExitStack()
        with cr:
            wp = cr.enter_context(tc.tile_pool(name="wp", bufs=1))
            sp = cr.enter_context(tc.tile_pool(name="sp", bufs=1))
            gxp = cr.enter_context(tc.tile_pool(name="gxp", bufs=3))
            ep = cr.enter_context(tc.tile_pool(name="ep", bufs=2))
            stp = cr.enter_context(tc.tile_pool(name="stp", bufs=1))
            pp = cr.enter_context(tc.tile_pool(name="pp", bufs=1, space="PSUM"))

            whh_sb = wp.tile([P, HC, 2 * 4 * H], BF)
            nc.sync.dma_start(whh_sb[:], whh)
            id_sb = wp.tile([16, 16], F32)
            nc.sync.dma_start(id_sb[:], ident)
            mask_sb = sp.tile([P, TP // UNROLL, UNROLL], F32)
            nc.sync.dma_start(
                mask_sb[:].rearrange("p j s -> p (j s)"), maskb)
            outs_sb = sp.tile([P, HC, TP // UNROLL, UNROLL], BF)
            nc.gpsimd.memset(outs_sb[:], 0.0)

            h_en = sp.tile([P, HC, 1], BF)
            h_cn = sp.tile([P, HC, 1], BF)
            c_st = sp.tile([P, HC], F32)
            nc.gpsimd.memset(h_en[:], 0.0)
            nc.gpsimd.memset(h_cn[:], 0.0)
            nc.gpsimd.memset(c_st[:], 0.0)

            gv = gxs.rearrange("(j s) p c -> j s p c", s=UNROLL)

            def step(jv, s):
                # gx_t/mask for step UNROLL*jv + s
                gxt_t = gxp.tile([P, 8 * HC], BF, tag="gx")
                nc.sync.dma_start(gxt_t[:], gv[ds(jv, 1)][0][s])
                gx_t = gxt_t[:]             # [P, 64]: [cn(32) | en(32)]
                mt = ep.tile([P, 1], F32, tag=f"mt{s}")
                nc.vector.tensor_copy(
                    mt[:],
                    mask_sb[:, ds(jv, 1), s:s + 1].rearrange(
                        "p a b -> p (a b)"))

                # ---- two matvecs on concurrent PE column groups ----
                # cn streams on col group 0 (psum partition 0), en on col
                # group 1 (psum partition 32).  Each cell accumulates its own
                # 8 k-chunks in its own psum region, so no cross-partition
                # fold is needed.  au holds [a=u+gx (64) | u (64)]; the
                # scatter DMAs land u directly in au's second half.
                au = ep.tile([P, 128], BF, tag="au")
                warm = pp.tile([1, 1], F32, tag="warm")
                for r in range(4):         # round r == psum bank r
                    cell, nh = r // 2, (r % 2) * 4   # chunks n = nh..nh+3
                    hbuf = (h_cn, h_en)[cell]
                    bank = pp.tile([P, 512], F32, tag=f"bank{r}")
                    for k in range(KPRUNE):
                        for g in range(4):   # col group g <-> psum part 32g
                            n = nh + g
                            nc.tensor.matmul(
                                bank[32 * g:32 * g + 1, :],
                                lhsT=hbuf[:, k, :],
                                rhs=whh_sb[:, k, cell * 4096 + n * 512:
                                           cell * 4096 + (n + 1) * 512],
                                start=(k == 0), stop=(k == KPRUNE - 1),
                                tile_position=(0, 32 * g))
                    # evictions compact the round's 4 chunks into a single
                    # staging row (partition-shift 32g -> 0, split DVE/ACT),
                    # then ONE scatter DMA per round lands them in au
                    st = stp.tile([1, 2048], BF, tag=f"str{r}")
                    stv = st[0:1, :].rearrange("o (p gm) -> o p gm", gm=16)
                    for g in range(4):
                        if not NO_EVICT:
                            src = bank[32 * g:32 * g + 1, :].rearrange(
                                "o (p m) -> o p m", m=4)
                            if g % 2 == 0:
                                nc.vector.tensor_copy(
                                    stv[:, :, 4 * g:4 * g + 4], src)
                            else:
                                nc.scalar.activation(
                                    stv[:, :, 4 * g:4 * g + 4], src, AF.Copy)
                    if not NO_DMA:
                        eng = nc.sync if r % 2 == 0 else nc.scalar
                        eng.dma_start(
                            au[:, 64 + 16 * r:80 + 16 * r], st[0:1, :])

                if NO_CHAIN:
                    return
                # ---- gate nonlinearities (all [128, x], fp32) ----
                # a = u + gx (token cells); au layout:
                #   [0:32]=a_en, [32:64]=a_cn, [64:96]=u_cn, [96:128]=u_en
                nc.vector.tensor_tensor(
                    out=au[:, 0:32], in0=au[:, 96:128],
                    in1=gx_t[:, 32:64], op=OP.add)
                nc.vector.tensor_tensor(
                    out=au[:, 32:64], in0=au[:, 64:96],
                    in1=gx_t[:, 0:32], op=OP.add)

                au_v = au[:].rearrange("p (q x) -> p q x", x=32)  # [128,4,32]
                sig = ep.tile([P, 4, 24], F32, tag="sig")
                tnh = ep.tile([P, 4, 8], F32, tag="tnh")
                nc.scalar.activation(sig[:], au_v[:, :, 0:24], AF.Sigmoid)
                nc.scalar.activation(tnh[:], au_v[:, :, 24:32], AF.Tanh)
                sa, sb_ = sig[:, 0:2, :], sig[:, 2:4, :]
                tga, tgb = tnh[:, 0:2, :], tnh[:, 2:4, :]

                def gsl(sx, g):  # gate slice g -> [128, 2, 8]
                    return sx[:, :, g * 8:(g + 1) * 8]

                c1 = ep.tile([P, 2, 8], F32, tag="c1")
                t1 = ep.tile([P, 2, 8], F32, tag="t1")
                nc.vector.tensor_tensor(out=t1[:], in0=gsl(sa, 0), in1=tga, op=OP.mult)
                nc.vector.tensor_tensor(out=c1[:, 0, :], in0=gsl(sa, 1)[:, 0, :], in1=c_st[:], op=OP.mult)
                nc.vector.tensor_tensor(out=c1[:, 1, :], in0=gsl(sa, 1)[:, 1, :], in1=c_st[:], op=OP.mult)
                nc.vector.tensor_tensor(out=c1[:], in0=c1[:], in1=t1[:], op=OP.add)
                th1 = ep.tile([P, 2, 8], F32, tag="th1")
                nc.scalar.activation(th1[:], c1[:], AF.Tanh)
                # t2/c2 run on DVE while ACT computes th1
                t2 = ep.tile([P, 2, 8], F32, tag="t2")
                c2 = ep.tile([P, 2, 8], F32, tag="c2")
                nc.vector.tensor_tensor(out=t2[:], in0=gsl(sb_, 0), in1=tgb, op=OP.mult)
                nc.vector.tensor_tensor(out=c2[:], in0=gsl(sb_, 1), in1=c1[:], op=OP.mult)
                nc.vector.tensor_tensor(out=c2[:], in0=c2[:], in1=t2[:], op=OP.add)
                th2 = ep.tile([P, 2, 8], F32, tag="th2")
                nc.scalar.activation(th2[:], c2[:], AF.Tanh)
                h1 = ep.tile([P, 2, 8], F32, tag="h1")   # [hA_en | hB_cn]
                nc.vector.tensor_tensor(out=h1[:], in0=gsl(sa, 2), in1=th1[:], op=OP.mult)
                h2 = ep.tile([P, 2, 8], F32, tag="h2")   # [hA_cn | hB_en]
                nc.vector.tensor_tensor(out=h2[:], in0=gsl(sb_, 2), in1=th2[:], op=OP.mult)
                nc.tensor.matmul(warm[:], lhsT=h2[:, 0, 0:1], rhs=h2[:, 0, 0:1],
                                 start=True, stop=True)

                # keep the PE's HAM activity window busy through the tail:
                # tiny matmuls tied to successive tail results so the
                # 4096-cycle activity window never sees a full idle period
                nc.tensor.matmul(warm[:], lhsT=au[:, 0:1], rhs=au[:, 0:1],
                                 start=True, stop=True)
                nc.tensor.matmul(warm[:], lhsT=sig[:, 0, 0:1], rhs=sig[:, 0, 0:1],
                                 start=True, stop=True)
                nc.tensor.matmul(warm[:], lhsT=c1[:, 0, 0:1], rhs=c1[:, 0, 0:1],
                                 start=True, stop=True)
                nc.tensor.matmul(warm[:], lhsT=th1[:, 0, 0:1], rhs=th1[:, 0, 0:1],
                                 start=True, stop=True)

                # ---- mask selects: out = m*A + (1-m)*B ----
                # h selects first: they gate the next step's matmul streams
                dd = ep.tile([P, 3, 8], F32, tag="dd")
                nc.vector.tensor_tensor(out=dd[:, 0, :], in0=h1[:, 0, :], in1=h2[:, 1, :], op=OP.subtract)
                nc.vector.tensor_tensor(out=dd[:, 1, :], in0=h2[:, 0, :], in1=h1[:, 1, :], op=OP.subtract)
                nc.vector.scalar_tensor_tensor(
                    out=h_en[:, :, 0], in0=dd[:, 0, :], scalar=mt[:], in1=h2[:, 1, :],
                    op0=OP.mult, op1=OP.add)
                nc.vector.scalar_tensor_tensor(
                    out=h_cn[:, :, 0], in0=dd[:, 1, :], scalar=mt[:], in1=h1[:, 1, :],
                    op0=OP.mult, op1=OP.add)
                nc.vector.tensor_tensor(out=dd[:, 2, :], in0=c2[:, 0, :], in1=c2[:, 1, :], op=OP.subtract)
                nc.vector.scalar_tensor_tensor(
                    out=c_st[:], in0=dd[:, 2, :], scalar=mt[:], in1=c2[:, 1, :],
                    op0=OP.mult, op1=OP.add)
                nc.vector.tensor_tensor(
                    out=outs_sb[:, :, ds(jv, 1), s:s + 1].rearrange(
                        "p h a b -> p h (a b)"),
                    in0=h_en[:], in1=h_cn[:], op=OP.add)

            def iteration(jv):
                for s in range(UNROLL):
                    step(jv, s)

            iters = (nsteps + UNROLL - 1) // UNROLL
            if iters > 1:
                with tc.For_i(0, iters) as iv:
                    iteration(iv)
            else:
                iteration(0)

            # dump outsT
            nc.sync.dma_start(outst.rearrange("(j p) t -> p j t", p=P), outs_sb[:])

    # ================= phase D: fc1 (hidT = relu(w1 @ outsT + b1)) ====
    if "D" not in SKIP:
      with tile.TileContext(nc) as tc:
        with ExitStack() as c3:
            bp = c3.enter_context(tc.tile_pool(name="bias1", bufs=1))
            b1_sb = bp.tile([P, HC], F32)
            nc.sync.dma_start(b1_sb[:], b1p)

            def relu_bias(nc_, psum, sbuf, md):
                mabs = md.m_tile_idx * md.m_subtiles + md.m_subtile_idx
                nc_.scalar.activation(sbuf[:], psum[:], AF.Relu,
                                      bias=b1_sb[:, mabs:mabs + 1])

            from concourse.kernels.tile_matmul import (
                composable_matmul_tile_kernel, dma_from_dram_kxm,
                dma_from_dram_kxn, dma_to_dram_mxn)
            kxm_pool = c3.enter_context(tc.tile_pool(name="kxm1", bufs=3))
            kxn_pool = c3.enter_context(tc.tile_pool(name="kxn1", bufs=3))
            kxm_producer, kxm_shape = dma_from_dram_kxm(kxm_pool, w1t)
            kxn_producer, kxn_shape = dma_from_dram_kxn(kxn_pool, outst)
            composable_matmul_tile_kernel(
                tc, kxm_shape, kxn_shape, hidt.dtype,
                kxm_producer, kxn_producer,
                mxn_consumer=dma_to_dram_mxn(hidt),
                mxn_subtile_reducer=relu_bias)

    # ================= phase E: fc2 (out = hidT.T @ b2) ========
    if "E" not in SKIP:
      with tile.TileContext(nc) as tc:
        with ExitStack() as c4:
            bp2 = c4.enter_context(tc.tile_pool(name="bias2", bufs=1))
            b2_sb = bp2.tile([P, VSP], F32)
            nc.sync.dma_start(b2_sb[:], b2p)

            def add_b2(nc_, sbuf, md, _):
                for si in range(sbuf.shape[1]):
                    nc_.vector.tensor_tensor(
                        out=sbuf[:, si, :], in0=sbuf[:, si, :],
                        in1=b2_sb[:, md.n_slice], op=OP.add)

            matmul_tile_kernel(
                tc,
                kxm_ap=hidt,          # [H, TP]
                kxn_ap=w2t,           # [H, VSP]
                mxn_ap=out,           # [TP, VSP]
                post_mxn_tile_fn=add_b2,
            )

    nc.compile()
    return nc


_CACHE = {}


def _get_nc(nsteps=NSTEPS, skip=None):
    key = (nsteps, tuple(sorted(skip)) if skip else None)
    if key not in _CACHE:
        _CACHE[key] = build(nsteps, skip)
    return _CACHE[key]


def prep_in_maps(sentence, mask, embedding, W_ih_en, W_hh_en, W_ih_cn, W_hh_cn,
                 fc_w1, fc_b1, fc_w2, fc_b2):
    sentence = np.asarray(sentence).astype(np.int32)
    mask = np.asarray(mask).astype(np.float32)
    embedding = np.asarray(embedding, np.float32)

    common = {
        "sent": sentence,
        "emb": embedding.astype(BF16),
        "whh": np.concatenate(
            [_pack_whh(np.asarray(W_hh_cn, np.float32)),
             _pack_whh(np.asarray(W_hh_en, np.float32))], axis=2),
        "wih": _pack_wih(W_ih_cn, W_ih_en),
        "maskb": np.broadcast_to(
            np.concatenate([mask, np.zeros(TP - T, np.float32)])[None, :],
            (P, TP)).copy(),
        "w1t": np.ascontiguousarray(np.asarray(fc_w1, np.float32).T).astype(BF16),
        "b1p": np.asarray(fc_b1, np.float32).reshape(HC, P).T.copy(),
        "ident": np.eye(16, dtype=np.float32),
    }
    in_maps = []
    for i in range(NCORES):
        w2s = np.zeros((H, VSP), BF16)
        w2s[:, :VS] = np.asarray(fc_w2, np.float32)[i * VS:(i + 1) * VS].T.astype(BF16)
        b2s = np.zeros((VSP,), np.float32)
        b2s[:VS] = np.asarray(fc_b2, np.float32)[i * VS:(i + 1) * VS]
        in_maps.append({**common, "w2t": w2s,
                        "b2p": np.broadcast_to(b2s[None, :], (P, VSP)).copy()})
    return in_maps


def kernel(**inputs):
    in_maps = prep_in_maps(**inputs)
    nc = _get_nc()
    res = run_bass_kernel_spmd(nc, in_maps, list(range(NCORES)))
    return np.concatenate([r["out"][:T, :VS] for r in res.results], axis=1)

